# revision 13
# baseline (speedup 1.0000x reference)
"""nn_AffSkLayer_83313775607973 — Trainium2 Bass/Tile kernel.

Full inputs -> full output. Sharding: 8 cores = (batch b in 0..3) x (t-half in 0..1).
Each core computes its (b, 4-t-slice) shard of the output; BatchNorm statistics
are combined with an 8-core AllReduce.

SPMD uniformity: per-core input is a 14-slice t-window
  slices 0..11: x[b, :, T0-4 .. T0+8) zero-filled outside [0,8)   (dwconv path)
  slice 12:     x[b, :, clamp(T0-1)]                              (prev-shift clamp)
  slice 13:     x[b, :, clamp(T0+4)]                              (next-shift clamp)
so one compiled program serves all 8 cores; all t-boundary handling is data.

Per-core pipeline (c=256, rc=64, n=784, own t = local 4..7):
  xc = W_dc2 @ x on the 6 affinity slices {12,4,5,6,7,13}; nx = xc/||xc||_c.
  9 affinity Grams (4 self + 5 cross, cross shared between prev/next shifts);
  exp on ScalarE with fused row-sums (accum_out); col-sums and diagonals via
  PE ones-matmuls; softmax-diagonal factors staged in DRAM and broadcast to
  128 partitions by DMA. feats = sum of 3 diag-scaled shifted xc.
  xx = (W_dc@W_dc2) @ x on slices 0..11; the 3 dilated depthwise 9x3x3 convs
  run as per-tap scalar_tensor_tensor FMAs on a (c,tg)-packed padded window.
  up2in = relu(W_back/3 @ agg) * feats (one STT); y = x + W_up2 @ up2in;
  BN sums AllReduced; out = relu(gamma*(y-mean)*rstd + beta).
"""

import numpy as np
import ml_dtypes

BF16 = ml_dtypes.bfloat16
TEMP = 0.07
BN_EPS = 1e-5
C_EXP = float(np.exp(np.float32(1.0 / TEMP)))

B, C, T, H, W = 4, 256, 8, 28, 28
N = H * W
RC = 64
TH = 4
NOWN = TH * N
NCORES = 8

_CACHE = {}


# ---------------------------------------------------------------- host prep --
def _prep_inputs(x, W_dc2, W_up2, W_dc, W_sa1, W_sa2, W_sa3, W_back, gamma, beta):
    x = np.asarray(x, np.float32)

    def lhsT_pack(Wm):  # (co, ci) -> (128, nk, co) bf16, lhsT chunks on free dim
        t = np.ascontiguousarray(np.asarray(Wm, np.float32).T)  # (ci, co)
        ci, co = t.shape
        nk = ci // 128
        return np.ascontiguousarray(
            t.reshape(nk, 128, co).transpose(1, 0, 2).reshape(128, nk * co)
        ).astype(BF16)

    Wf = np.asarray(W_dc, np.float32) @ np.asarray(W_dc2, np.float32)  # (rc, c)
    lw = {
        "w_dc2": lhsT_pack(W_dc2),
        "w_f": lhsT_pack(Wf),
        "w_up2": lhsT_pack(np.asarray(W_up2, np.float32) / 3.0),
        "w_back": np.ascontiguousarray(
            (np.asarray(W_back, np.float32) / 3.0).T
        ).astype(BF16),
        "ones128": np.ones((128, 1), BF16),
    }
    taps = []
    for Wsa in (W_sa1, W_sa2, W_sa3):
        taps.append(np.asarray(Wsa, np.float32)[:, 0].reshape(64, 81))
    dw = np.concatenate(taps, axis=1)  # (64, 243)
    lw["dw_w"] = np.ascontiguousarray(np.concatenate([dw, dw], axis=0))  # (128,243)
    lw["gamma2"] = np.ascontiguousarray(
        np.asarray(gamma, np.float32).reshape(2, 128).T
    )
    lw["beta2"] = np.ascontiguousarray(
        np.asarray(beta, np.float32).reshape(2, 128).T
    )

    in_maps = []
    for k in range(NCORES):
        b_idx, th = k // 2, k % 2
        T0 = TH * th
        xb = x[b_idx].reshape(C, T, N)
        win = np.zeros((C, 14, N), np.float32)
        lo, hi = T0 - 4, T0 + 8
        vlo, vhi = max(lo, 0), min(hi, T)
        win[:, vlo - lo : vhi - lo, :] = xb[:, vlo:vhi, :]
        win[:, 12, :] = xb[:, max(T0 - 1, 0), :]
        win[:, 13, :] = xb[:, min(T0 + TH, T - 1), :]
        m = dict(lw)
        m["xw"] = np.ascontiguousarray(win.reshape(C, 14 * N)).astype(BF16)
        m["xf32"] = np.ascontiguousarray(
            xb[:, T0 : T0 + TH, :].reshape(C, NOWN)
        ).astype(np.float32)
        in_maps.append(m)
    return in_maps


# ------------------------------------------------------------- kernel build --
def _build():
    import concourse.bass as bass
    import concourse.tile as tile
    import concourse.mybir as mybir
    from contextlib import ExitStack

    f32 = mybir.dt.float32
    bf16 = mybir.dt.bfloat16
    AT = mybir.ActivationFunctionType
    OP = mybir.AluOpType

    nc = bass.Bass()
    xw_d = nc.declare_dram_parameter("xw", [C, 14 * N], bf16, isOutput=False)
    xf_d = nc.declare_dram_parameter("xf32", [C, NOWN], f32, isOutput=False)
    wdc2_d = nc.declare_dram_parameter("w_dc2", [128, 2 * 256], bf16, isOutput=False)
    wf_d = nc.declare_dram_parameter("w_f", [128, 2 * 64], bf16, isOutput=False)
    wup2_d = nc.declare_dram_parameter("w_up2", [128, 2 * 256], bf16, isOutput=False)
    wback_d = nc.declare_dram_parameter("w_back", [64, 256], bf16, isOutput=False)
    dww_d = nc.declare_dram_parameter("dw_w", [128, 243], f32, isOutput=False)
    ones_d = nc.declare_dram_parameter("ones128", [128, 1], bf16, isOutput=False)
    gam_d = nc.declare_dram_parameter("gamma2", [128, 2], f32, isOutput=False)
    bet_d = nc.declare_dram_parameter("beta2", [128, 2], f32, isOutput=False)
    y_d = nc.declare_dram_parameter("y", [C, NOWN], f32, isOutput=True)

    dsm_d = nc.dram_tensor("dsm_pack", [TH, 5, N], f32)
    invn_d = nc.dram_tensor("invn_pack", [6, N], f32)
    ccin_d = nc.dram_tensor("cc_in", [128, 4], f32)
    ccout_d = nc.dram_tensor("cc_out", [128, 4], f32)

    NT = [(0, 512), (512, 272)]
    AFF = 6
    S_AFF = [12, 4, 5, 6, 7, 13]

    with tile.TileContext(nc) as tc, ExitStack() as ctx:
        WP = ctx.enter_context(tc.tile_pool(name="wpool", bufs=1))
        ps = ctx.enter_context(tc.tile_pool(name="ps", bufs=4, space="PSUM"))
        dwp = ctx.enter_context(tc.tile_pool(name="dwp", bufs=1))
        fp = ctx.enter_context(tc.tile_pool(name="fp", bufs=1))
        ctx2 = ctx.enter_context(ExitStack())
        affp = ctx2.enter_context(tc.tile_pool(name="affp", bufs=1))
        wk = ctx2.enter_context(tc.tile_pool(name="wk", bufs=2))

        # ---- weights / constants
        wdc2 = WP.tile([128, 2, 256], bf16, tag="wdc2")
        nc.sync.dma_start(wdc2, wdc2_d.rearrange("p (k m) -> p k m", k=2))
        wf = WP.tile([128, 2, 64], bf16, tag="wf")
        nc.sync.dma_start(wf, wf_d.rearrange("p (k m) -> p k m", k=2))
        wup2 = WP.tile([128, 2, 256], bf16, tag="wup2")
        nc.sync.dma_start(wup2, wup2_d.rearrange("p (k m) -> p k m", k=2))
        wback = WP.tile([64, 256], bf16, tag="wback")
        nc.sync.dma_start(wback, wback_d[:])
        dww = WP.tile([128, 243], f32, tag="dww")
        nc.sync.dma_start(dww, dww_d[:])
        ones = WP.tile([128, 1], bf16, tag="ones")
        nc.sync.dma_start(ones, ones_d[:])
        gam = WP.tile([128, 2], f32, tag="gam")
        nc.sync.dma_start(gam, gam_d[:])
        bet = WP.tile([128, 2], f32, tag="bet")
        nc.sync.dma_start(bet, bet_d[:])

        xc = [affp.tile([128, AFF, N], bf16, tag=f"xc{ct}", name=f"xc{ct}") for ct in range(2)]
        nx = [affp.tile([128, AFF, N], bf16, tag=f"nx{ct}", name=f"nx{ct}") for ct in range(2)]
        feats = [fp.tile([128, TH, N], bf16, tag=f"feats{ct}", name=f"feats{ct}") for ct in range(2)]
        win = dwp.tile([128, 10, 34 * 34], bf16, tag="win")
        acc = dwp.tile([128, 2, 28, 28], bf16, tag="acc")

        # =================== phase 0: dc2 + xx matmuls ========================
        with tc.tile_pool(name="xp", bufs=1) as XP:
            xw = [XP.tile([128, 14, N], bf16, tag=f"xw{ct}", name=f"xw{ct}") for ct in range(2)]
            for ct in range(2):
                nc.sync.dma_start(
                    xw[ct],
                    xw_d[128 * ct : 128 * (ct + 1)].rearrange(
                        "p (s n) -> p s n", s=14
                    ),
                )
            xxpad = XP.tile([64, 12, 34 * 34], bf16, tag="xxpad")
            nc.gpsimd.memset(xxpad, 0.0)

            for a, s in enumerate(S_AFF):
                for ct in range(2):
                    pt = ps.tile([128, N], f32, tag="ps")
                    for n0, nw in NT:
                        for kt in range(2):
                            nc.tensor.matmul(
                                pt[:, n0 : n0 + nw],
                                lhsT=wdc2[:, kt, 128 * ct : 128 * (ct + 1)],
                                rhs=xw[kt][:, s, n0 : n0 + nw],
                                start=(kt == 0),
                                stop=(kt == 1),
                            )
                    nc.vector.tensor_copy(xc[ct][:, a, :], pt)

            xxv = xxpad.rearrange("p t (h w) -> p t h w", h=34)
            for s in range(12):
                pt = ps.tile([128, N], f32, tag="ps")
                for n0, nw in NT:
                    for kt in range(2):
                        nc.tensor.matmul(
                            pt[:64, n0 : n0 + nw],
                            lhsT=wf[:, kt, :],
                            rhs=xw[kt][:, s, n0 : n0 + nw],
                            start=(kt == 0),
                            stop=(kt == 1),
                        )
                nc.vector.tensor_copy(
                    xxv[:, s, 3:31, 3:31],
                    pt[:64, :].rearrange("p (h w) -> p h w", h=28),
                )

            # build the two t-group windows for the depthwise conv
            for tg in range(2):
                nc.sync.dma_start(
                    win[64 * tg : 64 * (tg + 1)],
                    xxpad[:, 2 * tg : 2 * tg + 10, :],
                )

        # =================== phase 1: norms -> nx =============================
        for a in range(AFF):
            sq = wk.tile([128, 2, N], bf16, tag="sq")
            for ct in range(2):
                nc.vector.tensor_mul(sq[:, ct, :], xc[ct][:, a, :], xc[ct][:, a, :])
            pt = ps.tile([1, N], f32, tag="ps")
            for n0, nw in NT:
                for ct in range(2):
                    nc.tensor.matmul(
                        pt[:, n0 : n0 + nw],
                        lhsT=ones,
                        rhs=sq[:, ct, n0 : n0 + nw],
                        start=(ct == 0),
                        stop=(ct == 1),
                    )
            lnv = wk.tile([1, N], f32, tag="lnv")
            nc.vector.tensor_scalar_max(lnv, pt, 1e-30)
            nc.scalar.activation(lnv, lnv, AT.Ln)
            nc.scalar.activation(lnv, lnv, AT.Exp, scale=-0.5)
            nc.sync.dma_start(invn_d[a : a + 1, :], lnv)
            invb = wk.tile([128, N], bf16, tag="invb")
            _iap = invn_d[:]
            bc = bass.AP(
                tensor=_iap.tensor,
                offset=_iap.offset + a * N,
                ap=[[0, 128], [1, N]],
            )
            nc.gpsimd.dma_start(invb, bc)
            for ct in range(2):
                nc.vector.tensor_mul(nx[ct][:, a, :], xc[ct][:, a, :], invb)

        # =================== phase 2: affinity ================================
        def gram(lhs_a, rhs_a, rowsum_to, do_colsum):
            cs = ps.tile([1, N], f32, tag="ps", name="cs") if do_colsum else None
            for mt in range(7):
                pt = ps.tile([112, N], f32, tag="ps")
                for n0, nw in NT:
                    for kt in range(2):
                        nc.tensor.matmul(
                            pt[:, n0 : n0 + nw],
                            lhsT=nx[kt][:, lhs_a, 112 * mt : 112 * (mt + 1)],
                            rhs=nx[kt][:, rhs_a, n0 : n0 + nw],
                            start=(kt == 0),
                            stop=(kt == 1),
                        )
                et = wk.tile([112, N], bf16, tag="et")
                nc.scalar.activation(
                    et, pt, AT.Exp, scale=1.0 / TEMP,
                    accum_out=rowsum_to[:, mt : mt + 1],
                )
                if do_colsum:
                    for n0, nw in NT:
                        nc.tensor.matmul(
                            cs[:, n0 : n0 + nw],
                            lhsT=ones[:112],
                            rhs=et[:, n0 : n0 + nw],
                            start=(mt == 0),
                            stop=(mt == 6),
                            skip_group_check=True,
                        )
            return cs

        def rr_to_dram(rr, t_idx, slot):
            _dap = dsm_d[:]
            dst = bass.AP(
                tensor=_dap.tensor,
                offset=_dap.offset + (t_idx * 5 + slot) * N,
                ap=[[1, 112], [112, 7]],
            )
            nc.sync.dma_start(dst, rr)

        for t in range(4, 8):  # self matrices
            a = t - 3
            rs = wk.tile([112, 7], f32, tag="rs")
            gram(a, a, rs, False)
            rr = wk.tile([112, 7], f32, tag="rr")
            nc.vector.reciprocal(rr, rs)
            nc.vector.tensor_scalar_mul(rr, rr, C_EXP)
            rr_to_dram(rr, t - 4, 0)

        for p in range(4, 9):  # cross matrices
            rs = wk.tile([112, 7], f32, tag="rs")
            cs = gram(p - 3, p - 4, rs, True)
            dm = wk.tile([128, 2, N], bf16, tag="sq")
            for ct in range(2):
                nc.vector.tensor_mul(
                    dm[:, ct, :], nx[ct][:, p - 3, :], nx[ct][:, p - 4, :]
                )
            dgp = ps.tile([1, N], f32, tag="ps")
            for n0, nw in NT:
                for ct in range(2):
                    nc.tensor.matmul(
                        dgp[:, n0 : n0 + nw],
                        lhsT=ones,
                        rhs=dm[:, ct, n0 : n0 + nw],
                        start=(ct == 0),
                        stop=(ct == 1),
                    )
            ed = wk.tile([1, N], f32, tag="ed", bufs=1)
            nc.scalar.activation(ed, dgp, AT.Exp, scale=1.0 / TEMP)
            if p <= 7:
                nc.sync.dma_start(dsm_d[p - 4 : p - 3, 3:4, :], ed)
                rr = wk.tile([112, 7], f32, tag="rr")
                nc.vector.reciprocal(rr, rs)
                rr_to_dram(rr, p - 4, 1)
            if p >= 5:
                nc.sync.dma_start(dsm_d[p - 5 : p - 4, 4:5, :], ed)
                rc1 = wk.tile([1, N], f32, tag="rc1", bufs=1)
                nc.vector.reciprocal(rc1, cs)
                nc.sync.dma_start(dsm_d[p - 5 : p - 4, 2:3, :], rc1)

        # =================== phase 3: feats ===================================
        for t in range(4, 8):
            dsmb = wk.tile([128, 5, N], bf16, tag="dsmb")
            _dap2 = dsm_d[:]
            bc = bass.AP(
                tensor=_dap2.tensor,
                offset=_dap2.offset + (t - 4) * 5 * N,
                ap=[[0, 128], [1, 5 * N]],
            )
            nc.gpsimd.dma_start(dsmb.rearrange("p a n -> p (a n)"), bc)
            aS, aP, aN = t - 3, t - 4, t - 2
            for ct in range(2):
                f1 = wk.tile([128, N], bf16, tag="f1")
                nc.vector.tensor_mul(f1, xc[ct][:, aP, :], dsmb[:, 1, :])
                nc.vector.tensor_mul(f1, f1, dsmb[:, 3, :])
                f2 = wk.tile([128, N], bf16, tag="f2")
                nc.vector.tensor_mul(f2, xc[ct][:, aN, :], dsmb[:, 2, :])
                nc.vector.tensor_mul(f2, f2, dsmb[:, 4, :])
                f3 = wk.tile([128, N], bf16, tag="f3")
                nc.vector.tensor_mul(f3, xc[ct][:, aS, :], dsmb[:, 0, :])
                nc.gpsimd.tensor_add(f1, f1, f2)
                nc.gpsimd.tensor_add(feats[ct][:, t - 4, :], f1, f3)

        ctx2.close()  # release affinity-phase SBUF (xc, nx, work tiles)

        # =================== phase 4: depthwise conv ==========================
        wv4 = win.rearrange("p t (h w) -> p t h w", h=34)
        ti = 0
        for d in (1, 2, 3):
            off = 3 - d  # buffer has fixed 3-px border; conv pad is d
            for kt in range(9):
                for ky in range(3):
                    for kx in range(3):
                        for j in range(2):
                            tap = wv4[
                                :, kt + j,
                                ky * d + off : ky * d + off + 28,
                                kx * d + off : kx * d + off + 28,
                            ]
                            if ti == 0:
                                nc.vector.tensor_scalar_mul(
                                    acc[:, j], tap, dww[:, 0:1]
                                )
                            else:
                                nc.vector.scalar_tensor_tensor(
                                    acc[:, j], tap, dww[:, ti : ti + 1],
                                    acc[:, j], op0=OP.mult, op1=OP.add,
                                )
                        ti += 1

        # =================== phase 5: score*feats, up2, y =====================
        late = ctx.enter_context(tc.tile_pool(name="late", bufs=1))
        wkl = ctx.enter_context(tc.tile_pool(name="wkl", bufs=2))
        y = [late.tile([128, NOWN], f32, tag=f"y{ct}", name=f"y{ct}") for ct in range(2)]
        accf = acc.rearrange("p t h w -> p (t h w)")  # (128, 1568)
        acc1 = late.tile([64, 1568], bf16, tag="acc1")
        nc.gpsimd.tensor_copy(acc1, accf[64:128, :])
        NT2 = [(0, 512), (512, 512), (1024, 512), (1536, 32)]
        for tg in range(2):
            rhs_src = accf if tg == 0 else acc1
            for ct in range(2):
                fv = feats[ct].rearrange("p t n -> p (t n)")
                for n0, nw in NT2:
                    pt = ps.tile([128, 512], f32, tag="ps")
                    nc.tensor.matmul(
                        pt[:, :nw],
                        lhsT=wback[:, 128 * ct : 128 * (ct + 1)],
                        rhs=rhs_src[0:64, n0 : n0 + nw],
                        start=True,
                        stop=True,
                    )
                    o0 = 1568 * tg + n0
                    nc.vector.scalar_tensor_tensor(
                        fv[:, o0 : o0 + nw],
                        pt[:, :nw], 0.0, fv[:, o0 : o0 + nw],
                        op0=OP.max, op1=OP.mult,
                    )

        for ct in range(2):
            nc.sync.dma_start(y[ct], xf_d[128 * ct : 128 * (ct + 1)])
        ysum = WP.tile([128, 2, 7], f32, tag="ysum")
        ysq = WP.tile([128, 2, 7], f32, tag="ysq")
        NT3 = [(i * 512, min(512, NOWN - i * 512)) for i in range(7)]
        for ct in range(2):
            for i, (n0, nw) in enumerate(NT3):
                pt = ps.tile([128, 512], f32, tag="ps")
                for kt in range(2):
                    nc.tensor.matmul(
                        pt[:, :nw],
                        lhsT=wup2[:, kt, 128 * ct : 128 * (ct + 1)],
                        rhs=feats[kt].rearrange("p t n -> p (t n)")[:, n0 : n0 + nw],
                        start=(kt == 0),
                        stop=(kt == 1),
                    )
                nc.vector.scalar_tensor_tensor(
                    y[ct][:, n0 : n0 + nw],
                    y[ct][:, n0 : n0 + nw], 1.0, pt[:, :nw],
                    op0=OP.mult, op1=OP.add,
                    accum_out=ysum[:, ct, i : i + 1],
                )

        # =================== phase 6: BN allreduce + apply ====================
        for ct in range(2):
            for i, (n0, nw) in enumerate(NT3):
                scr = wkl.tile([128, 512], bf16, tag="scr")
                nc.vector.scalar_tensor_tensor(
                    scr[:, :nw],
                    y[ct][:, n0 : n0 + nw], 1.0, y[ct][:, n0 : n0 + nw],
                    op0=OP.mult, op1=OP.mult,
                    accum_out=ysq[:, ct, i : i + 1],
                )
        stat = WP.tile([128, 4], f32, tag="stat")
        for ct in range(2):
            nc.vector.tensor_reduce(
                stat[:, 2 * ct : 2 * ct + 1], ysum[:, ct, :],
                axis=mybir.AxisListType.X, op=OP.add,
            )
            nc.vector.tensor_reduce(
                stat[:, 2 * ct + 1 : 2 * ct + 2], ysq[:, ct, :],
                axis=mybir.AxisListType.X, op=OP.add,
            )
        nc.sync.dma_start(ccin_d[:], stat)
        nc.gpsimd.collective_compute(
            "AllReduce",
            OP.add,
            replica_groups=[list(range(NCORES))],
            ins=[ccin_d[:]],
            outs=[ccout_d[:]],
        )
        gstat = WP.tile([128, 4], f32, tag="gstat")
        nc.sync.dma_start(gstat, ccout_d[:])

        M = float(B * T * N)
        for ct in range(2):
            mean = WP.tile([128, 1], f32, tag=f"mean{ct}")
            nc.scalar.activation(
                mean, gstat[:, 2 * ct : 2 * ct + 1], AT.Copy, scale=1.0 / M
            )
            msq = wkl.tile([128, 1], f32, tag="msq")
            nc.vector.tensor_mul(msq, mean, mean)
            var = wkl.tile([128, 1], f32, tag="var")
            nc.vector.scalar_tensor_tensor(
                var, gstat[:, 2 * ct + 1 : 2 * ct + 2], 1.0 / M, msq,
                op0=OP.mult, op1=OP.subtract,
            )
            nc.vector.tensor_scalar_add(var, var, BN_EPS)
            nc.scalar.activation(var, var, AT.Ln)
            rstd = wkl.tile([128, 1], f32, tag="rstd")
            nc.scalar.activation(rstd, var, AT.Exp, scale=-0.5)
            sc = WP.tile([128, 1], f32, tag=f"sc{ct}")
            nc.vector.tensor_mul(sc, gam[:, ct : ct + 1], rstd)
            sh = WP.tile([128, 1], f32, tag=f"sh{ct}")
            nc.vector.tensor_mul(sh, mean, sc)
            nc.vector.scalar_tensor_tensor(
                sh, sh, -1.0, bet[:, ct : ct + 1], op0=OP.mult, op1=OP.add,
            )
            nc.scalar.activation(y[ct], y[ct], AT.Relu, bias=sh, scale=sc)
            nc.sync.dma_start(y_d[128 * ct : 128 * (ct + 1)], y[ct])

    _split_waits(nc)
    return nc


def _split_waits(nc, max_waits=1):
    """Walrus in this container encodes at most ~1 sync-wait command per
    instruction ("Too many sync wait commands" otherwise). Hoist extra waits
    into same-engine NoOp prefixes, which wait-then-release in program order."""
    import concourse.mybir as mybir

    n = 0
    for f in nc.m.functions:
        for blk in f.blocks:
            out = []
            changed = False
            for inst in blk.instructions:
                si = inst.sync_info
                if si is not None and si.on_wait and len(si.on_wait) > max_waits:
                    waits = list(si.on_wait)
                    head, tail = waits[:-max_waits], waits[-max_waits:]
                    for w in head:
                        nop = mybir.InstNoOp(name=f"waitnop_{n}", ins=[], outs=[])
                        n += 1
                        nop.engine = inst.engine
                        nop.sync_info = mybir.SyncInfo(on_wait=[w], on_update=[])
                        out.append(nop)
                    inst.sync_info = mybir.SyncInfo(
                        on_wait=tail, on_update=list(si.on_update or [])
                    )
                    changed = True
                out.append(inst)
            if changed:
                blk.instructions = out
    return nc


def _get_nc():
    if "nc" not in _CACHE:
        _CACHE["nc"] = _build()
    return _CACHE["nc"]


# --------------------------------------------------------------- entry point --
def kernel(x, W_dc2, W_up2, W_dc, W_sa1, W_sa2, W_sa3, W_back, gamma, beta,
           _want_trace=False):
    from concourse.bass_utils import run_bass_kernel_spmd

    in_maps = _prep_inputs(x, W_dc2, W_up2, W_dc, W_sa1, W_sa2, W_sa3, W_back,
                           gamma, beta)
    nc = _get_nc()
    res = run_bass_kernel_spmd(nc, in_maps, core_ids=list(range(NCORES)),
                               trace=_want_trace)
    _CACHE["last_result"] = res
    out = np.empty((B, C, T, H, W), np.float32)
    for k in range(NCORES):
        b_idx, th = k // 2, k % 2
        yk = res.results[k]["y"].reshape(C, TH, H, W)
        out[b_idx, :, TH * th : TH * (th + 1)] = yk
    return out


# revision 16
# speedup vs baseline: 4955.9746x; 4955.9746x over previous
"""nn_AffSkLayer_83313775607973 — Trainium2 Bass/Tile kernel.

Full inputs -> full output. Sharding: 8 cores = (batch b in 0..3) x (t-half in 0..1).
Each core computes its (b, 4-t-slice) shard of the output; BatchNorm statistics
are combined with an 8-core AllReduce.

SPMD uniformity: per-core input is a 14-slice t-window
  slices 0..11: x[b, :, T0-4 .. T0+8) zero-filled outside [0,8)   (dwconv path)
  slice 12:     x[b, :, clamp(T0-1)]                              (prev-shift clamp)
  slice 13:     x[b, :, clamp(T0+4)]                              (next-shift clamp)
so one compiled program serves all 8 cores; all t-boundary handling is data.

Per-core pipeline (c=256, rc=64, n=784, own t = local 4..7):
  xc = W_dc2 @ x on the 6 affinity slices {12,4,5,6,7,13}; nx = xc/||xc||_c.
  9 affinity Grams (4 self + 5 cross, cross shared between prev/next shifts);
  exp on ScalarE with fused row-sums (accum_out); col-sums and diagonals via
  PE ones-matmuls; softmax-diagonal factors staged in DRAM and broadcast to
  128 partitions by DMA. feats = sum of 3 diag-scaled shifted xc.
  xx = (W_dc@W_dc2) @ x on slices 0..11; the 3 dilated depthwise 9x3x3 convs
  run as per-tap scalar_tensor_tensor FMAs on a (c,tg)-packed padded window.
  up2in = relu(W_back/3 @ agg) * feats (one STT); y = x + W_up2 @ up2in;
  BN sums AllReduced; out = relu(gamma*(y-mean)*rstd + beta).
"""

import numpy as np
import ml_dtypes

BF16 = ml_dtypes.bfloat16
TEMP = 0.07
BN_EPS = 1e-5
C_EXP = float(np.exp(np.float32(1.0 / TEMP)))

B, C, T, H, W = 4, 256, 8, 28, 28
N = H * W
RC = 64
TH = 4
NOWN = TH * N
NCORES = 8

_CACHE = {}


# ---------------------------------------------------------------- host prep --
def _prep_inputs(x, W_dc2, W_up2, W_dc, W_sa1, W_sa2, W_sa3, W_back, gamma, beta):
    x = np.asarray(x, np.float32)

    def lhsT_pack(Wm):  # (co, ci) -> (128, nk, co) bf16, lhsT chunks on free dim
        t = np.ascontiguousarray(np.asarray(Wm, np.float32).T)  # (ci, co)
        ci, co = t.shape
        nk = ci // 128
        return np.ascontiguousarray(
            t.reshape(nk, 128, co).transpose(1, 0, 2).reshape(128, nk * co)
        ).astype(BF16)

    Wf = np.asarray(W_dc, np.float32) @ np.asarray(W_dc2, np.float32)  # (rc, c)
    lw = {
        "w_dc2": lhsT_pack(W_dc2),
        "w_f": lhsT_pack(Wf),
        "w_up2": lhsT_pack(np.asarray(W_up2, np.float32) / 3.0),
        "w_back": np.ascontiguousarray(
            (np.asarray(W_back, np.float32) / 3.0).T
        ).astype(BF16),
        "ones128": np.ones((128, 1), BF16),
    }
    taps = []
    for Wsa in (W_sa1, W_sa2, W_sa3):
        taps.append(np.asarray(Wsa, np.float32)[:, 0].reshape(64, 81))
    dw = np.concatenate(taps, axis=1)  # (64, 243)
    lw["dw_w"] = np.ascontiguousarray(np.concatenate([dw, dw], axis=0))  # (128,243)
    lw["gamma2"] = np.ascontiguousarray(
        np.asarray(gamma, np.float32).reshape(2, 128).T
    )
    lw["beta2"] = np.ascontiguousarray(
        np.asarray(beta, np.float32).reshape(2, 128).T
    )

    in_maps = []
    for k in range(NCORES):
        b_idx, th = k // 2, k % 2
        T0 = TH * th
        xb = x[b_idx].reshape(C, T, N)
        win = np.zeros((C, 14, N), np.float32)
        lo, hi = T0 - 4, T0 + 8
        vlo, vhi = max(lo, 0), min(hi, T)
        win[:, vlo - lo : vhi - lo, :] = xb[:, vlo:vhi, :]
        win[:, 12, :] = xb[:, max(T0 - 1, 0), :]
        win[:, 13, :] = xb[:, min(T0 + TH, T - 1), :]
        m = dict(lw)
        m["xw"] = np.ascontiguousarray(win.reshape(C, 14 * N)).astype(BF16)
        m["xf32"] = np.ascontiguousarray(
            xb[:, T0 : T0 + TH, :].reshape(C, NOWN)
        ).astype(np.float32)
        in_maps.append(m)
    return in_maps


# ------------------------------------------------------------- kernel build --
def _build():
    import concourse.bass as bass
    import concourse.tile as tile
    import concourse.mybir as mybir
    from contextlib import ExitStack

    f32 = mybir.dt.float32
    bf16 = mybir.dt.bfloat16
    AT = mybir.ActivationFunctionType
    OP = mybir.AluOpType

    nc = bass.Bass()
    xw_d = nc.declare_dram_parameter("xw", [C, 14 * N], bf16, isOutput=False)
    xf_d = nc.declare_dram_parameter("xf32", [C, NOWN], f32, isOutput=False)
    wdc2_d = nc.declare_dram_parameter("w_dc2", [128, 2 * 256], bf16, isOutput=False)
    wf_d = nc.declare_dram_parameter("w_f", [128, 2 * 64], bf16, isOutput=False)
    wup2_d = nc.declare_dram_parameter("w_up2", [128, 2 * 256], bf16, isOutput=False)
    wback_d = nc.declare_dram_parameter("w_back", [64, 256], bf16, isOutput=False)
    dww_d = nc.declare_dram_parameter("dw_w", [128, 243], f32, isOutput=False)
    ones_d = nc.declare_dram_parameter("ones128", [128, 1], bf16, isOutput=False)
    gam_d = nc.declare_dram_parameter("gamma2", [128, 2], f32, isOutput=False)
    bet_d = nc.declare_dram_parameter("beta2", [128, 2], f32, isOutput=False)
    y_d = nc.declare_dram_parameter("y", [C, NOWN], f32, isOutput=True)

    dsm_d = nc.dram_tensor("dsm_pack", [TH, 5, N], f32)
    invn_d = nc.dram_tensor("invn_pack", [6, N], f32)
    ccin_d = nc.dram_tensor("cc_in", [128, 4], f32)
    ccout_d = nc.dram_tensor("cc_out", [128, 4], f32)

    NT = [(0, 512), (512, 272)]
    AFF = 6
    S_AFF = [12, 4, 5, 6, 7, 13]

    with tile.TileContext(nc) as tc, ExitStack() as ctx:
        WP = ctx.enter_context(tc.tile_pool(name="wpool", bufs=1))
        ps = ctx.enter_context(tc.tile_pool(name="ps", bufs=4, space="PSUM"))
        dwp = ctx.enter_context(tc.tile_pool(name="dwp", bufs=1))
        fp = ctx.enter_context(tc.tile_pool(name="fp", bufs=1))
        ctx2 = ctx.enter_context(ExitStack())
        affp = ctx2.enter_context(tc.tile_pool(name="affp", bufs=1))
        wk = ctx2.enter_context(tc.tile_pool(name="wk", bufs=2))

        # ---- weights / constants
        wdc2 = WP.tile([128, 2, 256], bf16, tag="wdc2")
        nc.sync.dma_start(wdc2, wdc2_d.rearrange("p (k m) -> p k m", k=2))
        wf = WP.tile([128, 2, 64], bf16, tag="wf")
        nc.sync.dma_start(wf, wf_d.rearrange("p (k m) -> p k m", k=2))
        wup2 = WP.tile([128, 2, 256], bf16, tag="wup2")
        nc.sync.dma_start(wup2, wup2_d.rearrange("p (k m) -> p k m", k=2))
        wback = WP.tile([64, 256], bf16, tag="wback")
        nc.sync.dma_start(wback, wback_d[:])
        dww = WP.tile([128, 243], f32, tag="dww")
        nc.sync.dma_start(dww, dww_d[:])
        ones = WP.tile([128, 1], bf16, tag="ones")
        nc.sync.dma_start(ones, ones_d[:])
        gam = WP.tile([128, 2], f32, tag="gam")
        nc.sync.dma_start(gam, gam_d[:])
        bet = WP.tile([128, 2], f32, tag="bet")
        nc.sync.dma_start(bet, bet_d[:])

        xc = [affp.tile([128, AFF, N], bf16, tag=f"xc{ct}", name=f"xc{ct}") for ct in range(2)]
        nx = [affp.tile([128, AFF, N], bf16, tag=f"nx{ct}", name=f"nx{ct}") for ct in range(2)]
        feats = [fp.tile([128, TH, N], bf16, tag=f"feats{ct}", name=f"feats{ct}") for ct in range(2)]
        win = dwp.tile([128, 10, 34 * 34], bf16, tag="win")
        acc = dwp.tile([128, 2, 28, 28], bf16, tag="acc")

        # =================== phase 0: dc2 + xx matmuls ========================
        with tc.tile_pool(name="xp", bufs=1) as XP:
            xw = [XP.tile([128, 14, N], bf16, tag=f"xw{ct}", name=f"xw{ct}") for ct in range(2)]
            for ct in range(2):
                nc.sync.dma_start(
                    xw[ct],
                    xw_d[128 * ct : 128 * (ct + 1)].rearrange(
                        "p (s n) -> p s n", s=14
                    ),
                )
            xxpad = XP.tile([64, 12, 34 * 34], bf16, tag="xxpad")
            nc.gpsimd.memset(xxpad, 0.0)

            for a, s in enumerate(S_AFF):
                for ct in range(2):
                    pt = ps.tile([128, N], f32, tag="ps")
                    for n0, nw in NT:
                        for kt in range(2):
                            nc.tensor.matmul(
                                pt[:, n0 : n0 + nw],
                                lhsT=wdc2[:, kt, 128 * ct : 128 * (ct + 1)],
                                rhs=xw[kt][:, s, n0 : n0 + nw],
                                start=(kt == 0),
                                stop=(kt == 1),
                            )
                    nc.vector.tensor_copy(xc[ct][:, a, :], pt)

            xxv = xxpad.rearrange("p t (h w) -> p t h w", h=34)
            for s in range(12):
                pt = ps.tile([128, N], f32, tag="ps")
                for n0, nw in NT:
                    for kt in range(2):
                        nc.tensor.matmul(
                            pt[:64, n0 : n0 + nw],
                            lhsT=wf[:, kt, :],
                            rhs=xw[kt][:, s, n0 : n0 + nw],
                            start=(kt == 0),
                            stop=(kt == 1),
                        )
                nc.vector.tensor_copy(
                    xxv[:, s, 3:31, 3:31],
                    pt[:64, :].rearrange("p (h w) -> p h w", h=28),
                )

            # build the two t-group windows for the depthwise conv
            for tg in range(2):
                nc.sync.dma_start(
                    win[64 * tg : 64 * (tg + 1)],
                    xxpad[:, 2 * tg : 2 * tg + 10, :],
                )

        # =================== phase 1: norms -> nx =============================
        for a in range(AFF):
            sq = wk.tile([128, 2, N], bf16, tag="sq")
            for ct in range(2):
                nc.vector.tensor_mul(sq[:, ct, :], xc[ct][:, a, :], xc[ct][:, a, :])
            pt = ps.tile([1, N], f32, tag="ps")
            for n0, nw in NT:
                for ct in range(2):
                    nc.tensor.matmul(
                        pt[:, n0 : n0 + nw],
                        lhsT=ones,
                        rhs=sq[:, ct, n0 : n0 + nw],
                        start=(ct == 0),
                        stop=(ct == 1),
                    )
            lnv = wk.tile([1, N], f32, tag="lnv")
            nc.vector.tensor_scalar_max(lnv, pt, 1e-30)
            nc.scalar.activation(lnv, lnv, AT.Ln)
            nc.scalar.activation(lnv, lnv, AT.Exp, scale=-0.5)
            nc.sync.dma_start(invn_d[a : a + 1, :], lnv)
            invb = wk.tile([128, N], bf16, tag="invb")
            _iap = invn_d[:]
            bc = bass.AP(
                tensor=_iap.tensor,
                offset=_iap.offset + a * N,
                ap=[[0, 128], [1, N]],
            )
            nc.gpsimd.dma_start(invb, bc)
            for ct in range(2):
                nc.vector.tensor_mul(nx[ct][:, a, :], xc[ct][:, a, :], invb)

        # =================== phase 2: affinity ================================
        def gram(lhs_a, rhs_a, rowsum_to, do_colsum):
            cs = ps.tile([1, N], f32, tag="ps", name="cs") if do_colsum else None
            for mt in range(7):
                pt = ps.tile([112, N], f32, tag="ps")
                for n0, nw in NT:
                    for kt in range(2):
                        nc.tensor.matmul(
                            pt[:, n0 : n0 + nw],
                            lhsT=nx[kt][:, lhs_a, 112 * mt : 112 * (mt + 1)],
                            rhs=nx[kt][:, rhs_a, n0 : n0 + nw],
                            start=(kt == 0),
                            stop=(kt == 1),
                        )
                et = wk.tile([112, N], bf16, tag="et")
                nc.scalar.activation(
                    et, pt, AT.Exp, scale=1.0 / TEMP,
                    accum_out=rowsum_to[:, mt : mt + 1],
                )
                if do_colsum:
                    for n0, nw in NT:
                        nc.tensor.matmul(
                            cs[:, n0 : n0 + nw],
                            lhsT=ones[:112],
                            rhs=et[:, n0 : n0 + nw],
                            start=(mt == 0),
                            stop=(mt == 6),
                            skip_group_check=True,
                        )
            return cs

        def rr_to_dram(rr, t_idx, slot):
            _dap = dsm_d[:]
            dst = bass.AP(
                tensor=_dap.tensor,
                offset=_dap.offset + (t_idx * 5 + slot) * N,
                ap=[[1, 112], [112, 7]],
            )
            nc.sync.dma_start(dst, rr)

        for t in range(4, 8):  # self matrices
            a = t - 3
            rs = wk.tile([112, 7], f32, tag="rs")
            gram(a, a, rs, False)
            rr = wk.tile([112, 7], f32, tag="rr")
            nc.vector.reciprocal(rr, rs)
            nc.vector.tensor_scalar_mul(rr, rr, C_EXP)
            rr_to_dram(rr, t - 4, 0)

        for p in range(4, 9):  # cross matrices
            rs = wk.tile([112, 7], f32, tag="rs")
            cs = gram(p - 3, p - 4, rs, True)
            dm = wk.tile([128, 2, N], bf16, tag="sq")
            for ct in range(2):
                nc.vector.tensor_mul(
                    dm[:, ct, :], nx[ct][:, p - 3, :], nx[ct][:, p - 4, :]
                )
            dgp = ps.tile([1, N], f32, tag="ps")
            for n0, nw in NT:
                for ct in range(2):
                    nc.tensor.matmul(
                        dgp[:, n0 : n0 + nw],
                        lhsT=ones,
                        rhs=dm[:, ct, n0 : n0 + nw],
                        start=(ct == 0),
                        stop=(ct == 1),
                    )
            ed = wk.tile([1, N], f32, tag="ed", bufs=1)
            nc.scalar.activation(ed, dgp, AT.Exp, scale=1.0 / TEMP)
            if p <= 7:
                nc.sync.dma_start(dsm_d[p - 4 : p - 3, 3:4, :], ed)
                rr = wk.tile([112, 7], f32, tag="rr")
                nc.vector.reciprocal(rr, rs)
                rr_to_dram(rr, p - 4, 1)
            if p >= 5:
                nc.sync.dma_start(dsm_d[p - 5 : p - 4, 4:5, :], ed)
                rc1 = wk.tile([1, N], f32, tag="rc1", bufs=1)
                nc.vector.reciprocal(rc1, cs)
                nc.sync.dma_start(dsm_d[p - 5 : p - 4, 2:3, :], rc1)

        # =================== phase 3: feats ===================================
        for t in range(4, 8):
            dsmb = wk.tile([128, 5, N], bf16, tag="dsmb")
            _dap2 = dsm_d[:]
            bc = bass.AP(
                tensor=_dap2.tensor,
                offset=_dap2.offset + (t - 4) * 5 * N,
                ap=[[0, 128], [1, 5 * N]],
            )
            nc.gpsimd.dma_start(dsmb.rearrange("p a n -> p (a n)"), bc)
            aS, aP, aN = t - 3, t - 4, t - 2
            for ct in range(2):
                f1 = wk.tile([128, N], bf16, tag="f1")
                nc.vector.tensor_mul(f1, xc[ct][:, aP, :], dsmb[:, 1, :])
                nc.vector.tensor_mul(f1, f1, dsmb[:, 3, :])
                f2 = wk.tile([128, N], bf16, tag="f2")
                nc.vector.tensor_mul(f2, xc[ct][:, aN, :], dsmb[:, 2, :])
                nc.vector.tensor_mul(f2, f2, dsmb[:, 4, :])
                f3 = wk.tile([128, N], bf16, tag="f3")
                nc.vector.tensor_mul(f3, xc[ct][:, aS, :], dsmb[:, 0, :])
                nc.gpsimd.tensor_add(f1, f1, f2)
                nc.gpsimd.tensor_add(feats[ct][:, t - 4, :], f1, f3)

        ctx2.close()  # release affinity-phase SBUF (xc, nx, work tiles)

        # =================== phase 4: depthwise conv ==========================
        wk2 = ctx.enter_context(tc.tile_pool(name="wk2", bufs=4))
        wv4 = win.rearrange("p t (h w) -> p t h w", h=34)
        ti = 0
        first = {0: True, 1: True}
        for d in (1, 2, 3):
            off = 3 - d  # buffer has fixed 3-px border; conv pad is d
            for kt in range(9):
                for ky in range(3):
                    for kx in range(3):
                        # ~70% of taps: ScalarE does the strided scale, DVE
                        # the contiguous accumulate (2x mode); rest pure DVE.
                        on_act = (ti % 10) < 7
                        for j in range(2):
                            tap = wv4[
                                :, kt + j,
                                ky * d + off : ky * d + off + 28,
                                kx * d + off : kx * d + off + 28,
                            ]
                            if first[j]:
                                nc.vector.tensor_scalar_mul(
                                    acc[:, j], tap, dww[:, ti : ti + 1]
                                )
                                first[j] = False
                            elif on_act:
                                term = wk2.tile([128, 28, 28], bf16, tag="term")
                                nc.scalar.activation(
                                    term, tap, AT.Copy,
                                    scale=dww[:, ti : ti + 1],
                                )
                                nc.vector.tensor_add(acc[:, j], acc[:, j], term)
                            else:
                                nc.vector.scalar_tensor_tensor(
                                    acc[:, j], tap, dww[:, ti : ti + 1],
                                    acc[:, j], op0=OP.mult, op1=OP.add,
                                )
                        ti += 1

        # =================== phase 5: score*feats, up2, y =====================
        late = ctx.enter_context(tc.tile_pool(name="late", bufs=1))
        wkl = ctx.enter_context(tc.tile_pool(name="wkl", bufs=2))
        y = [late.tile([128, NOWN], f32, tag=f"y{ct}", name=f"y{ct}") for ct in range(2)]
        accf = acc.rearrange("p t h w -> p (t h w)")  # (128, 1568)
        acc1 = late.tile([64, 1568], bf16, tag="acc1")
        nc.gpsimd.tensor_copy(acc1, accf[64:128, :])
        NT2 = [(0, 512), (512, 512), (1024, 512), (1536, 32)]
        for tg in range(2):
            rhs_src = accf if tg == 0 else acc1
            for ct in range(2):
                fv = feats[ct].rearrange("p t n -> p (t n)")
                for n0, nw in NT2:
                    pt = ps.tile([128, 512], f32, tag="ps")
                    nc.tensor.matmul(
                        pt[:, :nw],
                        lhsT=wback[:, 128 * ct : 128 * (ct + 1)],
                        rhs=rhs_src[0:64, n0 : n0 + nw],
                        start=True,
                        stop=True,
                    )
                    o0 = 1568 * tg + n0
                    nc.vector.scalar_tensor_tensor(
                        fv[:, o0 : o0 + nw],
                        pt[:, :nw], 0.0, fv[:, o0 : o0 + nw],
                        op0=OP.max, op1=OP.mult,
                    )

        for ct in range(2):
            nc.sync.dma_start(y[ct], xf_d[128 * ct : 128 * (ct + 1)])
        ysum = WP.tile([128, 2, 7], f32, tag="ysum")
        ysq = WP.tile([128, 2, 7], f32, tag="ysq")
        NT3 = [(i * 512, min(512, NOWN - i * 512)) for i in range(7)]
        for ct in range(2):
            for i, (n0, nw) in enumerate(NT3):
                pt = ps.tile([128, 512], f32, tag="ps")
                for kt in range(2):
                    nc.tensor.matmul(
                        pt[:, :nw],
                        lhsT=wup2[:, kt, 128 * ct : 128 * (ct + 1)],
                        rhs=feats[kt].rearrange("p t n -> p (t n)")[:, n0 : n0 + nw],
                        start=(kt == 0),
                        stop=(kt == 1),
                    )
                nc.vector.scalar_tensor_tensor(
                    y[ct][:, n0 : n0 + nw],
                    y[ct][:, n0 : n0 + nw], 1.0, pt[:, :nw],
                    op0=OP.mult, op1=OP.add,
                    accum_out=ysum[:, ct, i : i + 1],
                )

        # =================== phase 6: BN allreduce + apply ====================
        for ct in range(2):
            for i, (n0, nw) in enumerate(NT3):
                scr = wkl.tile([128, 512], bf16, tag="scr")
                nc.vector.scalar_tensor_tensor(
                    scr[:, :nw],
                    y[ct][:, n0 : n0 + nw], 1.0, y[ct][:, n0 : n0 + nw],
                    op0=OP.mult, op1=OP.mult,
                    accum_out=ysq[:, ct, i : i + 1],
                )
        stat = WP.tile([128, 4], f32, tag="stat")
        for ct in range(2):
            nc.vector.tensor_reduce(
                stat[:, 2 * ct : 2 * ct + 1], ysum[:, ct, :],
                axis=mybir.AxisListType.X, op=OP.add,
            )
            nc.vector.tensor_reduce(
                stat[:, 2 * ct + 1 : 2 * ct + 2], ysq[:, ct, :],
                axis=mybir.AxisListType.X, op=OP.add,
            )
        nc.sync.dma_start(ccin_d[:], stat)
        nc.gpsimd.collective_compute(
            "AllReduce",
            OP.add,
            replica_groups=[list(range(NCORES))],
            ins=[ccin_d[:]],
            outs=[ccout_d[:]],
        )
        gstat = WP.tile([128, 4], f32, tag="gstat")
        nc.sync.dma_start(gstat, ccout_d[:])

        M = float(B * T * N)
        for ct in range(2):
            mean = WP.tile([128, 1], f32, tag=f"mean{ct}")
            nc.scalar.activation(
                mean, gstat[:, 2 * ct : 2 * ct + 1], AT.Copy, scale=1.0 / M
            )
            msq = wkl.tile([128, 1], f32, tag="msq")
            nc.vector.tensor_mul(msq, mean, mean)
            var = wkl.tile([128, 1], f32, tag="var")
            nc.vector.scalar_tensor_tensor(
                var, gstat[:, 2 * ct + 1 : 2 * ct + 2], 1.0 / M, msq,
                op0=OP.mult, op1=OP.subtract,
            )
            nc.vector.tensor_scalar_add(var, var, BN_EPS)
            nc.scalar.activation(var, var, AT.Ln)
            rstd = wkl.tile([128, 1], f32, tag="rstd")
            nc.scalar.activation(rstd, var, AT.Exp, scale=-0.5)
            sc = WP.tile([128, 1], f32, tag=f"sc{ct}")
            nc.vector.tensor_mul(sc, gam[:, ct : ct + 1], rstd)
            sh = WP.tile([128, 1], f32, tag=f"sh{ct}")
            nc.vector.tensor_mul(sh, mean, sc)
            nc.vector.scalar_tensor_tensor(
                sh, sh, -1.0, bet[:, ct : ct + 1], op0=OP.mult, op1=OP.add,
            )
            nc.scalar.activation(y[ct], y[ct], AT.Relu, bias=sh, scale=sc)
            nc.sync.dma_start(y_d[128 * ct : 128 * (ct + 1)], y[ct])

    _split_waits(nc)
    return nc


def _split_waits(nc, max_waits=1):
    """Walrus in this container encodes at most ~1 sync-wait command per
    instruction ("Too many sync wait commands" otherwise). Hoist extra waits
    into same-engine NoOp prefixes, which wait-then-release in program order."""
    import concourse.mybir as mybir

    n = 0
    for f in nc.m.functions:
        for blk in f.blocks:
            out = []
            changed = False
            for inst in blk.instructions:
                si = inst.sync_info
                if si is not None and si.on_wait and len(si.on_wait) > max_waits:
                    waits = list(si.on_wait)
                    head, tail = waits[:-max_waits], waits[-max_waits:]
                    for w in head:
                        nop = mybir.InstNoOp(name=f"waitnop_{n}", ins=[], outs=[])
                        n += 1
                        nop.engine = inst.engine
                        nop.sync_info = mybir.SyncInfo(on_wait=[w], on_update=[])
                        out.append(nop)
                    inst.sync_info = mybir.SyncInfo(
                        on_wait=tail, on_update=list(si.on_update or [])
                    )
                    changed = True
                out.append(inst)
            if changed:
                blk.instructions = out
    return nc


def _get_nc():
    if "nc" not in _CACHE:
        _CACHE["nc"] = _build()
    return _CACHE["nc"]


# --------------------------------------------------------------- entry point --
def kernel(x, W_dc2, W_up2, W_dc, W_sa1, W_sa2, W_sa3, W_back, gamma, beta,
           _want_trace=False):
    from concourse.bass_utils import run_bass_kernel_spmd

    in_maps = _prep_inputs(x, W_dc2, W_up2, W_dc, W_sa1, W_sa2, W_sa3, W_back,
                           gamma, beta)
    nc = _get_nc()
    res = run_bass_kernel_spmd(nc, in_maps, core_ids=list(range(NCORES)),
                               trace=_want_trace)
    _CACHE["last_result"] = res
    out = np.empty((B, C, T, H, W), np.float32)
    for k in range(NCORES):
        b_idx, th = k // 2, k % 2
        yk = res.results[k]["y"].reshape(C, TH, H, W)
        out[b_idx, :, TH * th : TH * (th + 1)] = yk
    return out


# revision 17
# speedup vs baseline: 5517.7656x; 1.1134x over previous
"""nn_AffSkLayer_83313775607973 — Trainium2 Bass/Tile kernel.

Full inputs -> full output. Sharding: 8 cores = (batch b in 0..3) x (t-half in 0..1).
Each core computes its (b, 4-t-slice) shard of the output; BatchNorm statistics
are combined with an 8-core AllReduce.

SPMD uniformity: per-core input is a 14-slice t-window
  slices 0..11: x[b, :, T0-4 .. T0+8) zero-filled outside [0,8)   (dwconv path)
  slice 12:     x[b, :, clamp(T0-1)]                              (prev-shift clamp)
  slice 13:     x[b, :, clamp(T0+4)]                              (next-shift clamp)
so one compiled program serves all 8 cores; all t-boundary handling is data.

Per-core pipeline (c=256, rc=64, n=784, own t = local 4..7):
  xc = W_dc2 @ x on the 6 affinity slices {12,4,5,6,7,13}; nx = xc/||xc||_c.
  9 affinity Grams (4 self + 5 cross, cross shared between prev/next shifts);
  exp on ScalarE with fused row-sums (accum_out); col-sums and diagonals via
  PE ones-matmuls; softmax-diagonal factors staged in DRAM and broadcast to
  128 partitions by DMA. feats = sum of 3 diag-scaled shifted xc.
  xx = (W_dc@W_dc2) @ x on slices 0..11; the 3 dilated depthwise 9x3x3 convs
  run as per-tap scalar_tensor_tensor FMAs on a (c,tg)-packed padded window.
  up2in = relu(W_back/3 @ agg) * feats (one STT); y = x + W_up2 @ up2in;
  BN sums AllReduced; out = relu(gamma*(y-mean)*rstd + beta).
"""

import numpy as np
import ml_dtypes

BF16 = ml_dtypes.bfloat16
TEMP = 0.07
BN_EPS = 1e-5
C_EXP = float(np.exp(np.float32(1.0 / TEMP)))

B, C, T, H, W = 4, 256, 8, 28, 28
N = H * W
RC = 64
TH = 4
NOWN = TH * N
NCORES = 8

_CACHE = {}


# ---------------------------------------------------------------- host prep --
def _prep_inputs(x, W_dc2, W_up2, W_dc, W_sa1, W_sa2, W_sa3, W_back, gamma, beta):
    x = np.asarray(x, np.float32)

    def lhsT_pack(Wm):  # (co, ci) -> (128, nk, co) bf16, lhsT chunks on free dim
        t = np.ascontiguousarray(np.asarray(Wm, np.float32).T)  # (ci, co)
        ci, co = t.shape
        nk = ci // 128
        return np.ascontiguousarray(
            t.reshape(nk, 128, co).transpose(1, 0, 2).reshape(128, nk * co)
        ).astype(BF16)

    Wf = np.asarray(W_dc, np.float32) @ np.asarray(W_dc2, np.float32)  # (rc, c)
    lw = {
        "w_dc2": lhsT_pack(W_dc2),
        "w_f": lhsT_pack(Wf),
        "w_up2": lhsT_pack(np.asarray(W_up2, np.float32) / 3.0),
        "w_back": np.ascontiguousarray(
            (np.asarray(W_back, np.float32) / 3.0).T
        ).astype(BF16),
        "ones128": np.ones((128, 1), BF16),
    }
    taps = []
    for Wsa in (W_sa1, W_sa2, W_sa3):
        taps.append(np.asarray(Wsa, np.float32)[:, 0].reshape(64, 81))
    dw = np.concatenate(taps, axis=1)  # (64, 243)
    lw["dw_w"] = np.ascontiguousarray(np.concatenate([dw, dw], axis=0))  # (128,243)
    lw["gamma2"] = np.ascontiguousarray(
        np.asarray(gamma, np.float32).reshape(2, 128).T
    )
    lw["beta2"] = np.ascontiguousarray(
        np.asarray(beta, np.float32).reshape(2, 128).T
    )

    in_maps = []
    for k in range(NCORES):
        b_idx, th = k // 2, k % 2
        T0 = TH * th
        xb = x[b_idx].reshape(C, T, N)
        win = np.zeros((C, 14, N), np.float32)
        lo, hi = T0 - 4, T0 + 8
        vlo, vhi = max(lo, 0), min(hi, T)
        win[:, vlo - lo : vhi - lo, :] = xb[:, vlo:vhi, :]
        win[:, 12, :] = xb[:, max(T0 - 1, 0), :]
        win[:, 13, :] = xb[:, min(T0 + TH, T - 1), :]
        m = dict(lw)
        m["xw"] = np.ascontiguousarray(win.reshape(C, 14 * N)).astype(BF16)
        m["xf32"] = np.ascontiguousarray(
            xb[:, T0 : T0 + TH, :].reshape(C, NOWN)
        ).astype(np.float32)
        in_maps.append(m)
    return in_maps


# ------------------------------------------------------------- kernel build --
def _build():
    import concourse.bass as bass
    import concourse.tile as tile
    import concourse.mybir as mybir
    from contextlib import ExitStack

    f32 = mybir.dt.float32
    bf16 = mybir.dt.bfloat16
    AT = mybir.ActivationFunctionType
    OP = mybir.AluOpType

    nc = bass.Bass()
    xw_d = nc.declare_dram_parameter("xw", [C, 14 * N], bf16, isOutput=False)
    xf_d = nc.declare_dram_parameter("xf32", [C, NOWN], f32, isOutput=False)
    wdc2_d = nc.declare_dram_parameter("w_dc2", [128, 2 * 256], bf16, isOutput=False)
    wf_d = nc.declare_dram_parameter("w_f", [128, 2 * 64], bf16, isOutput=False)
    wup2_d = nc.declare_dram_parameter("w_up2", [128, 2 * 256], bf16, isOutput=False)
    wback_d = nc.declare_dram_parameter("w_back", [64, 256], bf16, isOutput=False)
    dww_d = nc.declare_dram_parameter("dw_w", [128, 243], f32, isOutput=False)
    ones_d = nc.declare_dram_parameter("ones128", [128, 1], bf16, isOutput=False)
    gam_d = nc.declare_dram_parameter("gamma2", [128, 2], f32, isOutput=False)
    bet_d = nc.declare_dram_parameter("beta2", [128, 2], f32, isOutput=False)
    y_d = nc.declare_dram_parameter("y", [C, NOWN], f32, isOutput=True)

    dsm_d = nc.dram_tensor("dsm_pack", [TH, 5, N], f32)
    invn_d = nc.dram_tensor("invn_pack", [6, N], f32)
    ccin_d = nc.dram_tensor("cc_in", [128, 4], f32)
    ccout_d = nc.dram_tensor("cc_out", [128, 4], f32)

    NT = [(0, 512), (512, 272)]
    AFF = 6
    S_AFF = [12, 4, 5, 6, 7, 13]

    with tile.TileContext(nc) as tc, ExitStack() as ctx:
        WP = ctx.enter_context(tc.tile_pool(name="wpool", bufs=1))
        ps = ctx.enter_context(tc.tile_pool(name="ps", bufs=4, space="PSUM"))
        dwp = ctx.enter_context(tc.tile_pool(name="dwp", bufs=1))
        fp = ctx.enter_context(tc.tile_pool(name="fp", bufs=1))
        ctx2 = ctx.enter_context(ExitStack())
        affp = ctx2.enter_context(tc.tile_pool(name="affp", bufs=1))
        wk = ctx2.enter_context(tc.tile_pool(name="wk", bufs=2))

        # ---- weights / constants
        wdc2 = WP.tile([128, 2, 256], bf16, tag="wdc2")
        nc.sync.dma_start(wdc2, wdc2_d.rearrange("p (k m) -> p k m", k=2))
        wf = WP.tile([128, 2, 64], bf16, tag="wf")
        nc.sync.dma_start(wf, wf_d.rearrange("p (k m) -> p k m", k=2))
        wup2 = WP.tile([128, 2, 256], bf16, tag="wup2")
        nc.sync.dma_start(wup2, wup2_d.rearrange("p (k m) -> p k m", k=2))
        wback = WP.tile([64, 256], bf16, tag="wback")
        nc.sync.dma_start(wback, wback_d[:])
        dww = WP.tile([128, 243], f32, tag="dww")
        nc.sync.dma_start(dww, dww_d[:])
        ones = WP.tile([128, 1], bf16, tag="ones")
        nc.sync.dma_start(ones, ones_d[:])
        gam = WP.tile([128, 2], f32, tag="gam")
        nc.sync.dma_start(gam, gam_d[:])
        bet = WP.tile([128, 2], f32, tag="bet")
        nc.sync.dma_start(bet, bet_d[:])

        xc = [affp.tile([128, AFF, N], bf16, tag=f"xc{ct}", name=f"xc{ct}") for ct in range(2)]
        nx = [affp.tile([128, AFF, N], bf16, tag=f"nx{ct}", name=f"nx{ct}") for ct in range(2)]
        feats = [fp.tile([128, TH, N], bf16, tag=f"feats{ct}", name=f"feats{ct}") for ct in range(2)]
        win = dwp.tile([128, 10, 34 * 34], bf16, tag="win")
        acc = dwp.tile([128, 2, 28, 28], bf16, tag="acc")

        # =================== phase 0: dc2 + xx matmuls ========================
        with tc.tile_pool(name="xp", bufs=1) as XP:
            xw = [XP.tile([128, 14, N], bf16, tag=f"xw{ct}", name=f"xw{ct}") for ct in range(2)]
            for ct in range(2):
                nc.sync.dma_start(
                    xw[ct],
                    xw_d[128 * ct : 128 * (ct + 1)].rearrange(
                        "p (s n) -> p s n", s=14
                    ),
                )
            xxpad = XP.tile([64, 12, 34 * 34], bf16, tag="xxpad")
            nc.gpsimd.memset(xxpad, 0.0)

            for a, s in enumerate(S_AFF):
                for ct in range(2):
                    pt = ps.tile([128, N], f32, tag="ps")
                    for n0, nw in NT:
                        for kt in range(2):
                            nc.tensor.matmul(
                                pt[:, n0 : n0 + nw],
                                lhsT=wdc2[:, kt, 128 * ct : 128 * (ct + 1)],
                                rhs=xw[kt][:, s, n0 : n0 + nw],
                                start=(kt == 0),
                                stop=(kt == 1),
                            )
                    nc.vector.tensor_copy(xc[ct][:, a, :], pt)

            xxv = xxpad.rearrange("p t (h w) -> p t h w", h=34)
            for s in range(12):
                pt = ps.tile([128, N], f32, tag="ps")
                for n0, nw in NT:
                    for kt in range(2):
                        nc.tensor.matmul(
                            pt[:64, n0 : n0 + nw],
                            lhsT=wf[:, kt, :],
                            rhs=xw[kt][:, s, n0 : n0 + nw],
                            start=(kt == 0),
                            stop=(kt == 1),
                        )
                nc.vector.tensor_copy(
                    xxv[:, s, 3:31, 3:31],
                    pt[:64, :].rearrange("p (h w) -> p h w", h=28),
                )

            # build the two t-group windows for the depthwise conv
            for tg in range(2):
                nc.sync.dma_start(
                    win[64 * tg : 64 * (tg + 1)],
                    xxpad[:, 2 * tg : 2 * tg + 10, :],
                )

        # =================== phase 1: norms -> nx =============================
        for a in range(AFF):
            sq = wk.tile([128, 2, N], bf16, tag="sq")
            for ct in range(2):
                nc.vector.tensor_mul(sq[:, ct, :], xc[ct][:, a, :], xc[ct][:, a, :])
            pt = ps.tile([1, N], f32, tag="ps")
            for n0, nw in NT:
                for ct in range(2):
                    nc.tensor.matmul(
                        pt[:, n0 : n0 + nw],
                        lhsT=ones,
                        rhs=sq[:, ct, n0 : n0 + nw],
                        start=(ct == 0),
                        stop=(ct == 1),
                    )
            lnv = wk.tile([1, N], f32, tag="lnv")
            nc.vector.tensor_scalar_max(lnv, pt, 1e-30)
            nc.scalar.activation(lnv, lnv, AT.Ln)
            nc.scalar.activation(lnv, lnv, AT.Exp, scale=-0.5)
            nc.sync.dma_start(invn_d[a : a + 1, :], lnv)
            invb = wk.tile([128, N], bf16, tag="invb")
            _iap = invn_d[:]
            bc = bass.AP(
                tensor=_iap.tensor,
                offset=_iap.offset + a * N,
                ap=[[0, 128], [1, N]],
            )
            nc.gpsimd.dma_start(invb, bc)
            for ct in range(2):
                nc.vector.tensor_mul(nx[ct][:, a, :], xc[ct][:, a, :], invb)

        # =================== phase 2: affinity ================================
        def gram(lhs_a, rhs_a, rowsum_to, do_colsum):
            cs = ps.tile([1, N], f32, tag="ps", name="cs") if do_colsum else None
            for mt in range(7):
                pt = ps.tile([112, N], f32, tag="ps")
                for n0, nw in NT:
                    for kt in range(2):
                        nc.tensor.matmul(
                            pt[:, n0 : n0 + nw],
                            lhsT=nx[kt][:, lhs_a, 112 * mt : 112 * (mt + 1)],
                            rhs=nx[kt][:, rhs_a, n0 : n0 + nw],
                            start=(kt == 0),
                            stop=(kt == 1),
                        )
                et = wk.tile([112, N], bf16, tag="et")
                nc.scalar.activation(
                    et, pt, AT.Exp, scale=1.0 / TEMP,
                    accum_out=rowsum_to[:, mt : mt + 1],
                )
                if do_colsum:
                    for n0, nw in NT:
                        nc.tensor.matmul(
                            cs[:, n0 : n0 + nw],
                            lhsT=ones[:112],
                            rhs=et[:, n0 : n0 + nw],
                            start=(mt == 0),
                            stop=(mt == 6),
                            skip_group_check=True,
                        )
            return cs

        def rr_to_dram(rr, t_idx, slot):
            _dap = dsm_d[:]
            dst = bass.AP(
                tensor=_dap.tensor,
                offset=_dap.offset + (t_idx * 5 + slot) * N,
                ap=[[1, 112], [112, 7]],
            )
            nc.sync.dma_start(dst, rr)

        for t in range(4, 8):  # self matrices
            a = t - 3
            rs = wk.tile([112, 7], f32, tag="rs")
            gram(a, a, rs, False)
            rr = wk.tile([112, 7], f32, tag="rr")
            nc.vector.reciprocal(rr, rs)
            nc.vector.tensor_scalar_mul(rr, rr, C_EXP)
            rr_to_dram(rr, t - 4, 0)

        for p in range(4, 9):  # cross matrices
            rs = wk.tile([112, 7], f32, tag="rs")
            cs = gram(p - 3, p - 4, rs, True)
            dm = wk.tile([128, 2, N], bf16, tag="sq")
            for ct in range(2):
                nc.vector.tensor_mul(
                    dm[:, ct, :], nx[ct][:, p - 3, :], nx[ct][:, p - 4, :]
                )
            dgp = ps.tile([1, N], f32, tag="ps")
            for n0, nw in NT:
                for ct in range(2):
                    nc.tensor.matmul(
                        dgp[:, n0 : n0 + nw],
                        lhsT=ones,
                        rhs=dm[:, ct, n0 : n0 + nw],
                        start=(ct == 0),
                        stop=(ct == 1),
                    )
            ed = wk.tile([1, N], f32, tag="ed", bufs=1)
            nc.scalar.activation(ed, dgp, AT.Exp, scale=1.0 / TEMP)
            if p <= 7:
                nc.sync.dma_start(dsm_d[p - 4 : p - 3, 3:4, :], ed)
                rr = wk.tile([112, 7], f32, tag="rr")
                nc.vector.reciprocal(rr, rs)
                rr_to_dram(rr, p - 4, 1)
            if p >= 5:
                nc.sync.dma_start(dsm_d[p - 5 : p - 4, 4:5, :], ed)
                rc1 = wk.tile([1, N], f32, tag="rc1", bufs=1)
                nc.vector.reciprocal(rc1, cs)
                nc.sync.dma_start(dsm_d[p - 5 : p - 4, 2:3, :], rc1)

        # =================== phase 3: feats ===================================
        for t in range(4, 8):
            dsmb = wk.tile([128, 5, N], bf16, tag="dsmb")
            _dap2 = dsm_d[:]
            bc = bass.AP(
                tensor=_dap2.tensor,
                offset=_dap2.offset + (t - 4) * 5 * N,
                ap=[[0, 128], [1, 5 * N]],
            )
            nc.gpsimd.dma_start(dsmb.rearrange("p a n -> p (a n)"), bc)
            aS, aP, aN = t - 3, t - 4, t - 2
            for ct in range(2):
                f1 = wk.tile([128, N], bf16, tag="f1")
                nc.vector.tensor_mul(f1, xc[ct][:, aP, :], dsmb[:, 1, :])
                nc.vector.tensor_mul(f1, f1, dsmb[:, 3, :])
                f2 = wk.tile([128, N], bf16, tag="f2")
                nc.vector.tensor_mul(f2, xc[ct][:, aN, :], dsmb[:, 2, :])
                nc.vector.tensor_mul(f2, f2, dsmb[:, 4, :])
                f3 = wk.tile([128, N], bf16, tag="f3")
                nc.vector.tensor_mul(f3, xc[ct][:, aS, :], dsmb[:, 0, :])
                nc.gpsimd.tensor_add(f1, f1, f2)
                nc.gpsimd.tensor_add(feats[ct][:, t - 4, :], f1, f3)

        ctx2.close()  # release affinity-phase SBUF (xc, nx, work tiles)

        # =================== phase 4: depthwise conv ==========================
        wk2 = ctx.enter_context(tc.tile_pool(name="wk2", bufs=8))
        wv4 = win.rearrange("p t (h w) -> p t h w", h=34)
        # Two independent accumulator chains per j so the pure-DVE STT chain
        # never stalls behind ScalarE term production: acc (STT path) and
        # acc2 (ACT-scale -> DVE 2x-add path), summed at the end.
        acc2 = dwp.tile([128, 2, 28, 28], bf16, tag="acc2")
        ti = 0
        first = {("s", 0): True, ("s", 1): True, ("a", 0): True, ("a", 1): True}
        for d in (1, 2, 3):
            off = 3 - d  # buffer has fixed 3-px border; conv pad is d
            for kt in range(9):
                for ky in range(3):
                    for kx in range(3):
                        # ~70% of taps: ScalarE does the strided scale, DVE
                        # the contiguous accumulate (2x mode); rest pure DVE.
                        on_act = (ti % 10) < 7
                        for j in range(2):
                            tap = wv4[
                                :, kt + j,
                                ky * d + off : ky * d + off + 28,
                                kx * d + off : kx * d + off + 28,
                            ]
                            if on_act:
                                if first[("a", j)]:
                                    nc.scalar.activation(
                                        acc2[:, j], tap, AT.Copy,
                                        scale=dww[:, ti : ti + 1],
                                    )
                                    first[("a", j)] = False
                                else:
                                    term = wk2.tile(
                                        [128, 28, 28], bf16, tag="term"
                                    )
                                    nc.scalar.activation(
                                        term, tap, AT.Copy,
                                        scale=dww[:, ti : ti + 1],
                                    )
                                    nc.vector.tensor_add(
                                        acc2[:, j], acc2[:, j], term
                                    )
                            elif first[("s", j)]:
                                nc.vector.tensor_scalar_mul(
                                    acc[:, j], tap, dww[:, ti : ti + 1]
                                )
                                first[("s", j)] = False
                            else:
                                nc.vector.scalar_tensor_tensor(
                                    acc[:, j], tap, dww[:, ti : ti + 1],
                                    acc[:, j], op0=OP.mult, op1=OP.add,
                                )
                        ti += 1
        nc.vector.tensor_add(acc, acc, acc2)

        # =================== phase 5: score*feats, up2, y =====================
        late = ctx.enter_context(tc.tile_pool(name="late", bufs=1))
        wkl = ctx.enter_context(tc.tile_pool(name="wkl", bufs=2))
        y = [late.tile([128, NOWN], f32, tag=f"y{ct}", name=f"y{ct}") for ct in range(2)]
        accf = acc.rearrange("p t h w -> p (t h w)")  # (128, 1568)
        acc1 = late.tile([64, 1568], bf16, tag="acc1")
        nc.gpsimd.tensor_copy(acc1, accf[64:128, :])
        NT2 = [(0, 512), (512, 512), (1024, 512), (1536, 32)]
        for tg in range(2):
            rhs_src = accf if tg == 0 else acc1
            for ct in range(2):
                fv = feats[ct].rearrange("p t n -> p (t n)")
                for n0, nw in NT2:
                    pt = ps.tile([128, 512], f32, tag="ps")
                    nc.tensor.matmul(
                        pt[:, :nw],
                        lhsT=wback[:, 128 * ct : 128 * (ct + 1)],
                        rhs=rhs_src[0:64, n0 : n0 + nw],
                        start=True,
                        stop=True,
                    )
                    o0 = 1568 * tg + n0
                    nc.vector.scalar_tensor_tensor(
                        fv[:, o0 : o0 + nw],
                        pt[:, :nw], 0.0, fv[:, o0 : o0 + nw],
                        op0=OP.max, op1=OP.mult,
                    )

        for ct in range(2):
            nc.sync.dma_start(y[ct], xf_d[128 * ct : 128 * (ct + 1)])
        ysum = WP.tile([128, 2, 7], f32, tag="ysum")
        ysq = WP.tile([128, 2, 7], f32, tag="ysq")
        NT3 = [(i * 512, min(512, NOWN - i * 512)) for i in range(7)]
        for ct in range(2):
            for i, (n0, nw) in enumerate(NT3):
                pt = ps.tile([128, 512], f32, tag="ps")
                for kt in range(2):
                    nc.tensor.matmul(
                        pt[:, :nw],
                        lhsT=wup2[:, kt, 128 * ct : 128 * (ct + 1)],
                        rhs=feats[kt].rearrange("p t n -> p (t n)")[:, n0 : n0 + nw],
                        start=(kt == 0),
                        stop=(kt == 1),
                    )
                nc.vector.scalar_tensor_tensor(
                    y[ct][:, n0 : n0 + nw],
                    y[ct][:, n0 : n0 + nw], 1.0, pt[:, :nw],
                    op0=OP.mult, op1=OP.add,
                    accum_out=ysum[:, ct, i : i + 1],
                )

        # =================== phase 6: BN allreduce + apply ====================
        for ct in range(2):
            for i, (n0, nw) in enumerate(NT3):
                scr = wkl.tile([128, 512], bf16, tag="scr")
                nc.vector.scalar_tensor_tensor(
                    scr[:, :nw],
                    y[ct][:, n0 : n0 + nw], 1.0, y[ct][:, n0 : n0 + nw],
                    op0=OP.mult, op1=OP.mult,
                    accum_out=ysq[:, ct, i : i + 1],
                )
        stat = WP.tile([128, 4], f32, tag="stat")
        for ct in range(2):
            nc.vector.tensor_reduce(
                stat[:, 2 * ct : 2 * ct + 1], ysum[:, ct, :],
                axis=mybir.AxisListType.X, op=OP.add,
            )
            nc.vector.tensor_reduce(
                stat[:, 2 * ct + 1 : 2 * ct + 2], ysq[:, ct, :],
                axis=mybir.AxisListType.X, op=OP.add,
            )
        nc.sync.dma_start(ccin_d[:], stat)
        nc.gpsimd.collective_compute(
            "AllReduce",
            OP.add,
            replica_groups=[list(range(NCORES))],
            ins=[ccin_d[:]],
            outs=[ccout_d[:]],
        )
        gstat = WP.tile([128, 4], f32, tag="gstat")
        nc.sync.dma_start(gstat, ccout_d[:])

        M = float(B * T * N)
        for ct in range(2):
            mean = WP.tile([128, 1], f32, tag=f"mean{ct}")
            nc.scalar.activation(
                mean, gstat[:, 2 * ct : 2 * ct + 1], AT.Copy, scale=1.0 / M
            )
            msq = wkl.tile([128, 1], f32, tag="msq")
            nc.vector.tensor_mul(msq, mean, mean)
            var = wkl.tile([128, 1], f32, tag="var")
            nc.vector.scalar_tensor_tensor(
                var, gstat[:, 2 * ct + 1 : 2 * ct + 2], 1.0 / M, msq,
                op0=OP.mult, op1=OP.subtract,
            )
            nc.vector.tensor_scalar_add(var, var, BN_EPS)
            nc.scalar.activation(var, var, AT.Ln)
            rstd = wkl.tile([128, 1], f32, tag="rstd")
            nc.scalar.activation(rstd, var, AT.Exp, scale=-0.5)
            sc = WP.tile([128, 1], f32, tag=f"sc{ct}")
            nc.vector.tensor_mul(sc, gam[:, ct : ct + 1], rstd)
            sh = WP.tile([128, 1], f32, tag=f"sh{ct}")
            nc.vector.tensor_mul(sh, mean, sc)
            nc.vector.scalar_tensor_tensor(
                sh, sh, -1.0, bet[:, ct : ct + 1], op0=OP.mult, op1=OP.add,
            )
            nc.scalar.activation(y[ct], y[ct], AT.Relu, bias=sh, scale=sc)
            nc.sync.dma_start(y_d[128 * ct : 128 * (ct + 1)], y[ct])

    _split_waits(nc)
    return nc


def _split_waits(nc, max_waits=1):
    """Walrus in this container encodes at most ~1 sync-wait command per
    instruction ("Too many sync wait commands" otherwise). Hoist extra waits
    into same-engine NoOp prefixes, which wait-then-release in program order."""
    import concourse.mybir as mybir

    n = 0
    for f in nc.m.functions:
        for blk in f.blocks:
            out = []
            changed = False
            for inst in blk.instructions:
                si = inst.sync_info
                if si is not None and si.on_wait and len(si.on_wait) > max_waits:
                    waits = list(si.on_wait)
                    head, tail = waits[:-max_waits], waits[-max_waits:]
                    for w in head:
                        nop = mybir.InstNoOp(name=f"waitnop_{n}", ins=[], outs=[])
                        n += 1
                        nop.engine = inst.engine
                        nop.sync_info = mybir.SyncInfo(on_wait=[w], on_update=[])
                        out.append(nop)
                    inst.sync_info = mybir.SyncInfo(
                        on_wait=tail, on_update=list(si.on_update or [])
                    )
                    changed = True
                out.append(inst)
            if changed:
                blk.instructions = out
    return nc


def _get_nc():
    if "nc" not in _CACHE:
        _CACHE["nc"] = _build()
    return _CACHE["nc"]


# --------------------------------------------------------------- entry point --
def kernel(x, W_dc2, W_up2, W_dc, W_sa1, W_sa2, W_sa3, W_back, gamma, beta,
           _want_trace=False):
    from concourse.bass_utils import run_bass_kernel_spmd

    in_maps = _prep_inputs(x, W_dc2, W_up2, W_dc, W_sa1, W_sa2, W_sa3, W_back,
                           gamma, beta)
    nc = _get_nc()
    res = run_bass_kernel_spmd(nc, in_maps, core_ids=list(range(NCORES)),
                               trace=_want_trace)
    _CACHE["last_result"] = res
    out = np.empty((B, C, T, H, W), np.float32)
    for k in range(NCORES):
        b_idx, th = k // 2, k % 2
        yk = res.results[k]["y"].reshape(C, TH, H, W)
        out[b_idx, :, TH * th : TH * (th + 1)] = yk
    return out
